# revision 3
# baseline (speedup 1.0000x reference)
"""GATv2 (2-layer) Trainium2 Bass kernel, 8-core SPMD.

Strategy:
- Edges sorted by destination, grouped into 128-node dst blocks; contiguous
  block ranges assigned to the 8 cores with balanced edge counts.
- Per 128-edge tile: src features fetched with an indirect DMA row-gather
  from a full node table; dst features expanded from a streamed per-block
  node tile via a one-hot PE matmul (no dst gather).
- Segment softmax without max subtraction (exact: logits are O(1)); the
  denominator is accumulated with a ones-column matmul and applied after
  aggregation.
- leaky_relu(x, 0.2) == 0.6x + 0.4|x| lets logits be computed as two
  weighted row-reductions (no per-edge [H,C] activations materialized).
- Two launches (layer 1 -> host concat of per-core h slices -> layer 2).
"""

import json
import os
import sys
import time as _time
import numpy as np

_T0 = _time.time()


def _mark(msg):
    print(f"[kernel +{_time.time() - _T0:7.2f}s] {msg}", file=sys.stderr, flush=True)

import concourse.bass as bass
import concourse.mybir as mybir
from concourse.tile import TileContext, ScopedClock
from concourse.bass_utils import run_bass_kernel_spmd
from concourse.masks import make_identity

# ----------------------------------------------------------------------------
# Workarounds for the walrus build in this container: at most ONE sync-wait
# per instruction. Extra waits are peeled onto NoOps inserted just before.
# ----------------------------------------------------------------------------
_MAXW = 1
_split_counter = [0]


def _patched_drain_and_barrier(self, tick_clock, wait_clock):
    d0 = self.nc.sync.drain()
    wait_clock.add_sem_waits(d0.ins, ScopedClock({None: tick_clock.global_clock}))
    waits = list(d0.ins.sync_info.on_wait)
    if len(waits) > _MAXW:
        del d0.ins.sync_info.on_wait[_MAXW:]
        rest = waits[_MAXW:]
        for i in range(0, len(rest), _MAXW):
            d = self.nc.sync.drain()
            if d.ins.sync_info is None:
                d.ins.sync_info = mybir.SyncInfo(on_update=[], on_wait=[])
            d.ins.sync_info.on_wait.extend(rest[i:i + _MAXW])
    self.nc.all_engine_barrier()
    popped = self.nc._tile_sem_poison_stack.pop()
    assert popped is self._sem_poison
    self.nc.clear_and_free_semaphores(list(self.sems.allocated().values()))
    self.nc.all_engine_barrier()


def _fix_bir_json(data: bytes) -> bytes:
    m = json.loads(data)
    changed = False
    for f in m.get("functions", []):
        for b in f.get("blocks", []):
            insts = b.get("instructions")
            if not insts:
                continue
            out = []
            for inst in insts:
                si = inst.get("sync_info") or {}
                waits = si.get("on_wait") or []
                if len(waits) > 1:
                    for w in waits[:-1]:
                        _split_counter[0] += 1
                        out.append({
                            "name": f"I-sw{_split_counter[0]}",
                            "opcode": "NoOp",
                            "engine": inst.get("engine"),
                            "ins": [], "outs": [],
                            "sync_info": {"on_update": [], "on_wait": [w]},
                        })
                    si["on_wait"] = [waits[-1]]
                    changed = True
                out.append(inst)
            b["instructions"] = out
    if not changed:
        return data
    return json.dumps(m).encode()


def _install_fixes():
    TileContext._drain_and_barrier = _patched_drain_and_barrier
    if not getattr(bass.Bass, "_tilefix_json", False):
        orig = bass.Bass.to_json_bytes

        def to_json_bytes(self, *a, **k):
            return _fix_bir_json(orig(self, *a, **k))

        bass.Bass.to_json_bytes = to_json_bytes
        bass.Bass._tilefix_json = True


_install_fixes()

# ----------------------------------------------------------------------------
N_NODES = 100_000
N_EDGES = 1_600_000
F_IN = 128
H1, C1 = 2, 64
H2, C2 = 1, 64
NCORES = 8
P = 128
NBLK_GLOBAL = (N_NODES + P - 1) // P        # 782
NPAD = NBLK_GLOBAL * P                      # 100096
CH = 16                                     # stream chunk (tiles)
F32 = mybir.dt.float32
I32 = mybir.dt.int32


def _rep(v):
    """Replicate a 1-D row across 128 partitions -> [128, len] f32."""
    v = np.asarray(v, np.float32).reshape(1, -1)
    return np.ascontiguousarray(np.repeat(v, P, axis=0))


def _prep_edges(edge_index):
    src = np.asarray(edge_index[0], np.int64)
    dst = np.asarray(edge_index[1], np.int64)
    E = src.shape[0]
    order = np.argsort(dst, kind="stable")
    src_s = src[order].astype(np.int32)
    dst_s = dst[order].astype(np.int32)
    gb = dst_s // P                                    # global block id, sorted
    blk_cnt = np.bincount(gb, minlength=NBLK_GLOBAL)
    # contiguous block ranges per core, balanced by edge count
    cum = np.cumsum(blk_cnt)
    bounds = [0]
    for k in range(1, NCORES):
        t = E * k / NCORES
        b = int(np.searchsorted(cum, t))
        bounds.append(max(min(b, NBLK_GLOBAL - (NCORES - k)), bounds[-1] + 1))
    bounds.append(NBLK_GLOBAL)
    core_rng = [(bounds[k], bounds[k + 1]) for k in range(NCORES)]
    NBLKC = max(b1 - b0 for b0, b1 in core_rng)
    # tiles per block slot = max over cores
    T_list = []
    for s in range(NBLKC):
        mx = 1
        for b0, b1 in core_rng:
            if b0 + s < b1:
                mx = max(mx, (int(blk_cnt[b0 + s]) + P - 1) // P)
        T_list.append(mx + 1)  # leading all-pad tile per block (see _build_layer)
    T_total = sum(T_list)
    Tpad = ((T_total + CH - 1) // CH) * CH
    blk_start = np.concatenate([[0], cum]).astype(np.int64)  # edge offset per block
    idx_st = np.zeros((NCORES, P, Tpad), np.int32)
    ea_dummy = np.zeros((NCORES, P, Tpad), np.float32)
    dr_st = np.full((NCORES, P, Tpad), -1.0, np.float32)
    eorder = np.empty((NCORES,), object)
    slots = np.empty((NCORES,), object)
    for k in range(NCORES):
        b0, b1 = core_rng[k]
        col = 0
        ords, slts = [], []
        for s in range(NBLKC):
            b = b0 + s
            if b < b1:
                e0, e1 = int(blk_start[b]), int(blk_start[b + 1])
                n = e1 - e0
                sl = np.arange(n, dtype=np.int64)
                p_ = sl % P
                c_ = col + 1 + sl // P  # skip the leading pad tile
                idx_st[k, p_, c_] = src_s[e0:e1]
                dr_st[k, p_, c_] = (dst_s[e0:e1] % P).astype(np.float32)
                ords.append(order[e0:e1])
                slts.append((p_, c_))
            col += T_list[s]
        eorder[k] = ords
        slots[k] = slts
    return dict(core_rng=core_rng, NBLKC=NBLKC, T_list=T_list, Tpad=Tpad,
                idx_st=idx_st, dr_st=dr_st, ea_shape=ea_dummy.shape,
                eorder=eorder, slots=slots)


def _fill_ea(prep, edge_attr):
    ea = np.asarray(edge_attr, np.float32).reshape(-1)
    out = np.zeros((NCORES, P, prep["Tpad"]), np.float32)
    for k in range(NCORES):
        for (p_, c_), orig in zip(prep["slots"][k], prep["eorder"][k]):
            out[k, p_, c_] = ea[orig]
    return out


def _build_layer(COUT, H, NBLKC, T_list, Tpad, do_relu):
    """One GATv2 layer. Inputs (per core): xT [128,NPAD], xTl [128,NBLKC*128],
    streams idx/ea/dr, weights/consts. Output h_out [NBLKC*128, COUT]."""
    C = COUT // H
    nc = bass.Bass()
    xT = nc.dram_tensor("xT", [P, NPAD], F32, kind="ExternalInput")
    xTl = nc.dram_tensor("xTl", [P, NBLKC * P], F32, kind="ExternalInput")
    idx_d = nc.dram_tensor("idx", [P, Tpad], I32, kind="ExternalInput")
    ea_d = nc.dram_tensor("ea", [P, Tpad], F32, kind="ExternalInput")
    dr_d = nc.dram_tensor("dr", [P, Tpad], F32, kind="ExternalInput")
    Wl_d = nc.dram_tensor("Wl", [P, COUT], F32, kind="ExternalInput")
    Wr_d = nc.dram_tensor("Wr", [P, COUT], F32, kind="ExternalInput")
    vV_d = nc.dram_tensor("vV", [P, COUT], F32, kind="ExternalInput")
    attV_d = nc.dram_tensor("attV", [P, COUT], F32, kind="ExternalInput")
    biasV_d = nc.dram_tensor("biasV", [P, COUT], F32, kind="ExternalInput")
    bWlV_d = nc.dram_tensor("bWlV", [P, COUT], F32, kind="ExternalInput")
    bWrV_d = nc.dram_tensor("bWrV", [P, COUT], F32, kind="ExternalInput")
    iotaV_d = nc.dram_tensor("iotaV", [P, P], F32, kind="ExternalInput")
    onesV_d = nc.dram_tensor("onesV", [P, 1], F32, kind="ExternalInput")
    h_out = nc.dram_tensor("h_out", [NBLKC * P, COUT], F32, kind="ExternalOutput")
    xl_full = nc.dram_tensor("xl_full", [NPAD, COUT], F32)
    xr_loc = nc.dram_tensor("xr_loc", [NBLKC * P, COUT], F32)
    AL = mybir.AluOpType
    AF = mybir.ActivationFunctionType

    with TileContext(nc) as tc:
        with (
            tc.tile_pool(name="const", bufs=1) as cp,
            tc.tile_pool(name="sbuf", bufs=6) as pool,
            tc.tile_pool(name="st", bufs=3) as sp,
            tc.tile_pool(name="eps", bufs=2) as ep,
            tc.tile_pool(name="pd", bufs=2, space="PSUM") as ppd,
            tc.tile_pool(name="pt", bufs=2, space="PSUM") as ppt,
            tc.tile_pool(name="px", bufs=2, space="PSUM") as ppx,
            tc.tile_pool(name="po", bufs=2, space="PSUM") as ppo,
        ):
            Wl = cp.tile([P, COUT], F32); nc.sync.dma_start(out=Wl[:], in_=Wl_d[:])
            Wr = cp.tile([P, COUT], F32); nc.sync.dma_start(out=Wr[:], in_=Wr_d[:])
            vV = cp.tile([P, COUT], F32); nc.sync.dma_start(out=vV[:], in_=vV_d[:])
            attV = cp.tile([P, COUT], F32); nc.sync.dma_start(out=attV[:], in_=attV_d[:])
            biasV = cp.tile([P, COUT], F32); nc.sync.dma_start(out=biasV[:], in_=biasV_d[:])
            bWlV = cp.tile([P, COUT], F32); nc.sync.dma_start(out=bWlV[:], in_=bWlV_d[:])
            bWrV = cp.tile([P, COUT], F32); nc.sync.dma_start(out=bWrV[:], in_=bWrV_d[:])
            iotaV = cp.tile([P, P], F32); nc.sync.dma_start(out=iotaV[:], in_=iotaV_d[:])
            onesV = cp.tile([P, 1], F32); nc.sync.dma_start(out=onesV[:], in_=onesV_d[:])
            ident = cp.tile([P, P], F32); make_identity(nc, ident[:])
            Szero = cp.tile([P, P], F32)
            nc.vector.tensor_scalar(out=Szero[:], in0=iotaV[:], scalar1=0.0,
                                    scalar2=None, op0=mybir.AluOpType.mult)

            # dense: xl_full = (xT.T @ Wl) + b_l ; xr_loc likewise from xTl
            for j in range(NPAD // P):
                xt = pool.tile([P, P], F32, tag="xt")
                nc.sync.dma_start(out=xt[:], in_=xT[:, j * P:(j + 1) * P])
                pd = ppd.tile([P, COUT], F32, space="PSUM")
                nc.tensor.matmul(pd[:], lhsT=xt[:], rhs=Wl[:], start=True, stop=True)
                xls = pool.tile([P, COUT], F32, tag="xls")
                nc.vector.tensor_tensor(out=xls[:], in0=pd[:], in1=bWlV[:], op=AL.add)
                nc.sync.dma_start(out=xl_full[j * P:(j + 1) * P, :], in_=xls[:])
            for s in range(NBLKC):
                xt = pool.tile([P, P], F32, tag="xt")
                nc.sync.dma_start(out=xt[:], in_=xTl[:, s * P:(s + 1) * P])
                pd = ppd.tile([P, COUT], F32, space="PSUM")
                nc.tensor.matmul(pd[:], lhsT=xt[:], rhs=Wr[:], start=True, stop=True)
                xrs = pool.tile([P, COUT], F32, tag="xls")
                nc.vector.tensor_tensor(out=xrs[:], in0=pd[:], in1=bWrV[:], op=AL.add)
                nc.sync.dma_start(out=xr_loc[s * P:(s + 1) * P, :], in_=xrs[:])

            # edge phase
            g = 0
            for s in range(NBLKC):
                xrb = pool.tile([P, COUT], F32, tag="xrb")
                nc.sync.dma_start(out=xrb[:], in_=xr_loc[s * P:(s + 1) * P, :])
                psO = ppo.tile([P, COUT + H], F32, space="PSUM")
                for t in range(T_list[s]):
                    if g % CH == 0:
                        idxc = sp.tile([P, CH], I32, tag="idxc")
                        nc.sync.dma_start(out=idxc[:], in_=idx_d[:, g:g + CH])
                        eac = sp.tile([P, CH], F32, tag="eac")
                        nc.sync.dma_start(out=eac[:], in_=ea_d[:, g:g + CH])
                        drc = sp.tile([P, CH], F32, tag="drc")
                        nc.sync.dma_start(out=drc[:], in_=dr_d[:, g:g + CH])
                    c = g % CH
                    if t == 0:
                        # leading pad tile: absorbs the first start=True PSUM
                        # accumulation (its contribution is dropped by HW);
                        # zero matmuls only - no gather, no logit pipeline.
                        for h in range(H):
                            nc.tensor.matmul(psO[:, h * C:(h + 1) * C],
                                             lhsT=Szero[:],
                                             rhs=ident[:, :C],
                                             start=True, stop=False)
                            nc.tensor.matmul(psO[:, COUT + h:COUT + h + 1],
                                             lhsT=Szero[:], rhs=onesV[:],
                                             start=True, stop=False)
                        g += 1
                        continue
                    msgA = pool.tile([P, COUT], F32, tag="msgA")
                    nc.gpsimd.indirect_dma_start(
                        out=msgA[:], out_offset=None, in_=xl_full[:, :],
                        in_offset=bass.IndirectOffsetOnAxis(ap=idxc[:, c:c + 1], axis=0))
                    S01 = pool.tile([P, P], F32, tag="S01")
                    nc.vector.tensor_scalar(out=S01[:], in0=iotaV[:],
                                            scalar1=drc[:, c:c + 1], scalar2=None,
                                            op0=AL.is_equal)
                    pT = ppt.tile([P, P], F32, space="PSUM")
                    nc.tensor.transpose(out=pT[:], in_=S01[:], identity=ident[:])
                    selD = pool.tile([P, P], F32, tag="selD")
                    nc.scalar.copy(selD[:], pT[:])
                    pXR = ppx.tile([P, COUT], F32, space="PSUM")
                    nc.tensor.matmul(pXR[:], lhsT=selD[:], rhs=xrb[:], start=True, stop=True)
                    m = pool.tile([P, COUT], F32, tag="m")
                    nc.vector.scalar_tensor_tensor(out=m[:], in0=vV[:],
                                                   scalar=eac[:, c:c + 1], in1=pXR[:],
                                                   op0=AL.mult, op1=AL.add)
                    nc.vector.tensor_tensor(out=m[:], in0=m[:], in1=msgA[:], op=AL.add)
                    tabs = pool.tile([P, COUT], F32, tag="tabs")
                    nc.scalar.activation(tabs[:], m[:], AF.Abs)
                    q = pool.tile([P, COUT], F32, tag="q")
                    nc.vector.tensor_tensor(out=q[:], in0=m[:], in1=attV[:], op=AL.mult)
                    lin = pool.tile([P, H], F32, tag="lin")
                    nc.vector.tensor_reduce(out=lin[:], in_=q[:].rearrange("p (h c) -> p h c", h=H),
                                            axis=mybir.AxisListType.X, op=AL.add)
                    u = pool.tile([P, COUT], F32, tag="u")
                    nc.vector.tensor_tensor(out=u[:], in0=tabs[:], in1=attV[:], op=AL.mult)
                    ur = pool.tile([P, H], F32, tag="ur")
                    nc.vector.tensor_reduce(out=ur[:], in_=u[:].rearrange("p (h c) -> p h c", h=H),
                                            axis=mybir.AxisListType.X, op=AL.add)
                    logit = pool.tile([P, H], F32, tag="logit")
                    nc.vector.tensor_scalar(out=logit[:], in0=lin[:], scalar1=0.6,
                                            scalar2=None, op0=AL.mult)
                    nc.vector.scalar_tensor_tensor(out=logit[:], in0=ur[:], scalar=0.4,
                                                   in1=logit[:], op0=AL.mult, op1=AL.add)
                    ex = pool.tile([P, H], F32, tag="ex")
                    nc.scalar.activation(ex[:], logit[:], AF.Exp)
                    first, last = False, (t == T_list[s] - 1)
                    for h in range(H):
                        Sh = pool.tile([P, P], F32, tag=f"Sh{h}")
                        nc.scalar.activation(Sh[:], S01[:], AF.Copy,
                                             bias=0.0, scale=ex[:, h:h + 1])
                        nc.tensor.matmul(psO[:, h * C:(h + 1) * C], lhsT=Sh[:],
                                         rhs=msgA[:, h * C:(h + 1) * C],
                                         start=first, stop=last)
                        nc.tensor.matmul(psO[:, COUT + h:COUT + h + 1], lhsT=Sh[:],
                                         rhs=onesV[:], start=first, stop=last)
                    g += 1
                den = ep.tile([P, H], F32, tag="den")
                nc.vector.tensor_scalar_max(den[:], psO[:, COUT:COUT + H], 1e-30)
                dinv = ep.tile([P, H], F32, tag="dinv")
                nc.vector.reciprocal(dinv[:], den[:])
                hsb = ep.tile([P, COUT], F32, tag="hsb")
                for h in range(H):
                    nc.vector.tensor_scalar(out=hsb[:, h * C:(h + 1) * C],
                                            in0=psO[:, h * C:(h + 1) * C],
                                            scalar1=dinv[:, h:h + 1], scalar2=None,
                                            op0=AL.mult)
                nc.vector.tensor_tensor(out=hsb[:], in0=hsb[:], in1=biasV[:], op=AL.add)
                if do_relu:
                    nc.vector.tensor_scalar_max(hsb[:], hsb[:], 0.0)
                nc.sync.dma_start(out=h_out[s * P:(s + 1) * P, :], in_=hsb[:])
    return nc


def _run_layer(nc, per_core_ins):
    res = run_bass_kernel_spmd(nc, per_core_ins, core_ids=list(range(NCORES)))
    return [r["h_out"] for r in res.results]


def _layer_inputs(prep, ea_st, xT_full, Wl, bl, Wr, br, We, att, bias, COUT, H):
    iotaV = _rep(np.arange(P, dtype=np.float32))
    onesV = np.ones((P, 1), np.float32)
    common = dict(
        Wl=np.ascontiguousarray(Wl.astype(np.float32)),
        Wr=np.ascontiguousarray(Wr.astype(np.float32)),
        vV=_rep(We.reshape(-1)),
        attV=_rep(att.reshape(-1)),
        biasV=_rep(bias),
        bWlV=_rep(bl),
        bWrV=_rep(br),
        iotaV=iotaV,
        onesV=onesV,
        xT=xT_full,
    )
    per_core = []
    NBLKC = prep["NBLKC"]
    for k in range(NCORES):
        b0, b1 = prep["core_rng"][k]
        xTl = np.zeros((P, NBLKC * P), np.float32)
        lo, hi = b0 * P, min(b1 * P, NPAD)
        w = hi - lo
        xTl[:, :w] = xT_full[:, lo:hi]
        d = dict(common)
        d["xTl"] = xTl
        d["idx"] = prep["idx_st"][k]
        d["ea"] = ea_st[k]
        d["dr"] = prep["dr_st"][k]
        per_core.append(d)
    return per_core


def _assemble(prep, outs, COUT):
    full = np.zeros((NPAD, COUT), np.float32)
    for k in range(NCORES):
        b0, b1 = prep["core_rng"][k]
        n = (b1 - b0) * P
        full[b0 * P: b1 * P, :] = outs[k][:n, :]
    return full


def kernel(x, edge_index, edge_attr,
           W1_l, b1_l, W1_r, b1_r, W1_e, att1, bias1,
           W2_l, b2_l, W2_r, b2_r, W2_e, att2, bias2):
    _mark("kernel() start")
    x = np.asarray(x, np.float32)
    prep = _prep_edges(np.asarray(edge_index))
    _mark("prep_edges done")
    ea_st = _fill_ea(prep, edge_attr)
    _mark("fill_ea done")

    xpad = np.zeros((NPAD, F_IN), np.float32)
    xpad[:N_NODES] = x
    xT = np.ascontiguousarray(xpad.T)

    NBLKC, T_list, Tpad = prep["NBLKC"], prep["T_list"], prep["Tpad"]

    nc1 = _build_layer(H1 * C1, H1, NBLKC, T_list, Tpad, do_relu=True)
    _mark("build layer1 done")
    ins1 = _layer_inputs(prep, ea_st, xT, W1_l, b1_l, W1_r, b1_r, W1_e, att1,
                         bias1, H1 * C1, H1)
    _mark("layer1 inputs done")
    h_slices = _run_layer(nc1, ins1)
    _mark("run layer1 done")
    h_full = _assemble(prep, h_slices, H1 * C1)
    hT = np.ascontiguousarray(h_full.T)

    nc2 = _build_layer(H2 * C2, H2, NBLKC, T_list, Tpad, do_relu=False)
    _mark("build layer2 done")
    ins2 = _layer_inputs(prep, ea_st, hT, W2_l, b2_l, W2_r, b2_r, W2_e, att2,
                         bias2, H2 * C2, H2)
    _mark("layer2 inputs done")
    o_slices = _run_layer(nc2, ins2)
    _mark("run layer2 done")
    out_full = _assemble(prep, o_slices, H2 * C2)
    return out_full[:N_NODES].astype(np.float32)



# revision 39
# speedup vs baseline: 11.7060x; 11.7060x over previous
"""GATv2 (2-layer) Trainium2 Bass kernel, 8-core SPMD, single fused NEFF.

Wall-clock-oriented design (device exec is ~0.1s; build/compile/transfer
dominate):
- ONE kernel for both layers; h is exchanged on-device with an AllGather
  collective (no inter-layer host round trip, one compile, one launch).
- Uniform node sharding: 784 blocks of 128 nodes, 98 blocks per core, so
  AllGather slices concatenate into global node order and one edge-index
  stream serves both layers.
- xl/xr tables are row-interleaved ([2N, C]: row 2n = xl_n, 2n+1 = xr_n),
  so gathers use indices 2*src and 2*dst+1 into the same table.
- All per-core inputs are packed into a single f32 blob (one sharded
  transfer); int32 stream regions are viewed via AP.bitcast.
- Edge phase: per 128-edge tile only 5 instructions (2 gathers, one-hot
  build, exp-prescale into an rhs buffer that also carries the exp column,
  and ONE aggregation matmul over [cout+H] columns); the logit pipeline is
  batched over CH=32 tiles with broadcast APs.
- Segment softmax without max subtraction (logits are O(1); exact enough),
  denominator applied after aggregation. leaky_relu via 0.6x + 0.4|x|.
- Final output in bf16 (value-proportional rounding keeps relative error
  safe); inputs/tables stay f32 (bf16 there creates absolute-scale errors
  that blow up the relative-error metric at near-zero outputs).
"""

import json
import sys
import time as _time
import numpy as np

_T0 = _time.time()


def _mark(msg):
    print(f"[kernel +{_time.time() - _T0:6.2f}s] {msg}", file=sys.stderr, flush=True)

import concourse.bass as bass
import concourse.mybir as mybir
from concourse.tile import TileContext, ScopedClock
from concourse.bass_utils import run_bass_kernel_spmd
from concourse.masks import make_identity

# ----------------------------------------------------------------------------
# Workarounds for the walrus build in this container: at most ONE sync-wait
# per instruction. Extra waits are peeled onto NoOps inserted just before.
# ----------------------------------------------------------------------------
_MAXW = 1
_split_counter = [0]


def _patched_drain_and_barrier(self, tick_clock, wait_clock):
    d0 = self.nc.sync.drain()
    wait_clock.add_sem_waits(d0.ins, ScopedClock({None: tick_clock.global_clock}))
    waits = list(d0.ins.sync_info.on_wait)
    if len(waits) > _MAXW:
        del d0.ins.sync_info.on_wait[_MAXW:]
        rest = waits[_MAXW:]
        for i in range(0, len(rest), _MAXW):
            d = self.nc.sync.drain()
            if d.ins.sync_info is None:
                d.ins.sync_info = mybir.SyncInfo(on_update=[], on_wait=[])
            d.ins.sync_info.on_wait.extend(rest[i:i + _MAXW])
    self.nc.all_engine_barrier()
    popped = self.nc._tile_sem_poison_stack.pop()
    assert popped is self._sem_poison
    self.nc.clear_and_free_semaphores(list(self.sems.allocated().values()))
    self.nc.all_engine_barrier()


def _fix_bir_json(data: bytes) -> bytes:
    try:
        import orjson
        _loads, _dumps = orjson.loads, lambda m: orjson.dumps(m)
    except ImportError:
        _loads, _dumps = json.loads, lambda m: json.dumps(m).encode()
    m = _loads(data)
    changed = False
    for f in m.get("functions", []):
        for b in f.get("blocks", []):
            insts = b.get("instructions")
            if not insts:
                continue
            out = []
            for inst in insts:
                si = inst.get("sync_info") or {}
                waits = si.get("on_wait") or []
                if len(waits) > 1:
                    for w in waits[:-1]:
                        _split_counter[0] += 1
                        out.append({
                            "name": f"I-sw{_split_counter[0]}",
                            "opcode": "NoOp",
                            "engine": inst.get("engine"),
                            "ins": [], "outs": [],
                            "sync_info": {"on_update": [], "on_wait": [w]},
                        })
                    si["on_wait"] = [waits[-1]]
                    changed = True
                out.append(inst)
            b["instructions"] = out
    if not changed:
        return data
    return _dumps(m)


def _install_fixes():
    TileContext._drain_and_barrier = _patched_drain_and_barrier
    if not getattr(bass.Bass, "_tilefix_json", False):
        orig = bass.Bass.to_json_bytes

        def to_json_bytes(self, *a, **k):
            return _fix_bir_json(orig(self, *a, **k))

        bass.Bass.to_json_bytes = to_json_bytes
        bass.Bass._tilefix_json = True


_install_fixes()

# ----------------------------------------------------------------------------
N_NODES = 100_000
N_EDGES = 1_600_000
F_IN = 128
H1, C1 = 2, 64
H2, C2 = 1, 64
CO1, CO2 = H1 * C1, H2 * C2            # 128, 64
NCORES = 8
P = 128
NBLKC = 98                              # blocks per core
NLOC = NBLKC * P                        # 12544 nodes per core
NTOT = NCORES * NLOC                    # 100352 padded nodes
CH = 32                                 # tiles per merged logit chunk
F32 = mybir.dt.float32
BF16 = mybir.dt.bfloat16
I32 = mybir.dt.int32
AL = mybir.AluOpType
AF = mybir.ActivationFunctionType


def _rep(v):
    v = np.asarray(v, np.float32).reshape(1, -1)
    return np.ascontiguousarray(np.repeat(v, P, axis=0))


def _prep_edges(edge_index, edge_attr):
    """Sort edges by dst; build per-core [128, Tpad] streams (vectorized)."""
    src = np.asarray(edge_index[0], np.int64)
    dst = np.asarray(edge_index[1], np.int64)
    E = src.shape[0]
    order = np.argsort(dst, kind="stable")
    src_s = src[order].astype(np.int64)
    dst_s = dst[order].astype(np.int64)
    ea_s = np.asarray(edge_attr, np.float32).reshape(-1)[order]
    blk = (dst_s >> 7).astype(np.int64)            # global block 0..781
    cnt = np.bincount(blk, minlength=NCORES * NBLKC)
    T_slot = np.maximum((cnt.reshape(NCORES, NBLKC) + P - 1) // P, 1).max(axis=0)
    col0 = np.zeros(NBLKC + 1, np.int64)
    col0[1:] = np.cumsum(T_slot)
    sumT = int(col0[-1])
    Tpad = ((sumT + CH - 1) // CH) * CH
    T_slot = T_slot.copy()
    T_slot[-1] += Tpad - sumT                      # tail pad columns absorb
    runstart = np.zeros(NCORES * NBLKC + 1, np.int64)
    runstart[1:] = np.cumsum(cnt)
    rank = np.arange(E, dtype=np.int64) - runstart[blk]
    core = blk // NBLKC
    slot = blk - core * NBLKC
    col = col0[slot] + (rank >> 7)
    row = rank & 127

    idx_st = np.zeros((NCORES, P, Tpad), np.int32)
    dst_st = np.ones((NCORES, P, Tpad), np.int32)
    dr_st = np.full((NCORES, P, Tpad), -1.0, np.float32)
    ea_st = np.zeros((NCORES, P, Tpad), np.float32)
    idx_st[core, row, col] = (2 * src_s).astype(np.int32)
    dst_st[core, row, col] = (2 * dst_s + 1).astype(np.int32)
    dr_st[core, row, col] = (dst_s & 127).astype(np.float32)
    ea_st[core, row, col] = ea_s
    return dict(Tpad=Tpad, T_slot=T_slot.astype(np.int64), idx_st=idx_st,
                dst_st=dst_st, dr_st=dr_st, ea_st=ea_st)


def _build_kernel(Tpad, T_slot):
    nc = bass.Bass()

    # ---- blob layout (element offsets into the per-core [1, NW] f32 blob)
    widths = dict(iotaV=P, W1l=CO1, W1r=CO1, blr1=2 * CO1, vV1=CO1,
                  attV1=CO1, b1=CO1, W2l=CO2, W2r=CO2, blr2=2 * CO2,
                  vV2=CO2, attV2=CO2, b2=CO2)
    offs = {}
    off = 0
    for k, w in widths.items():
        offs[k] = off
        off += P * w
    offs["xT"] = off
    off += P * NLOC
    for s in ("idx", "dstg", "dr", "ea"):
        offs[s] = off
        off += P * Tpad
    NW = off

    blob = nc.dram_tensor("blob", [1, NW], F32, kind="ExternalInput")
    out_d = nc.dram_tensor("out", [NLOC, CO2], BF16, kind="ExternalOutput")
    XLR1_loc = nc.dram_tensor("XLR1_loc", [2 * NLOC, CO1], F32)
    XLR1 = nc.dram_tensor("XLR1", [2 * NTOT, CO1], F32)
    HL1 = nc.dram_tensor("HL1", [NLOC, CO1], F32)
    XLR2_loc = nc.dram_tensor("XLR2_loc", [2 * NLOC, CO2], F32)
    XLR2 = nc.dram_tensor("XLR2", [2 * NTOT, CO2], F32)

    def ap2d(name, w=None):
        o, tw = offs[name], widths.get(name, Tpad if name in
                                       ("idx", "dstg", "dr", "ea") else None)
        if name == "xT":
            tw = NLOC
        if w is None:
            w = tw
        return blob[0:1, o:o + P * tw].rearrange("o (p w) -> (o p) w", p=P)

    with TileContext(nc) as tc:
        with (
            tc.tile_pool(name="const", bufs=1) as cp,
            tc.tile_pool(name="dense", bufs=3) as dp,
            tc.tile_pool(name="st", bufs=3) as sp,
            tc.tile_pool(name="chunk", bufs=2) as chp,
            tc.tile_pool(name="tile", bufs=6) as tp,
            tc.tile_pool(name="ep", bufs=2) as epp,
            tc.tile_pool(name="pd", bufs=2, space="PSUM") as ppd,
            tc.tile_pool(name="po", bufs=2, space="PSUM") as ppo,
            tc.tile_pool(name="pt", bufs=2, space="PSUM") as ppt,
        ):
            C = {}
            for k, w in widths.items():
                t = cp.tile([P, w], F32, tag=f"c_{k}")
                nc.sync.dma_start(out=t[:], in_=ap2d(k))
                C[k] = t
            ident = cp.tile([P, P], F32)
            make_identity(nc, ident[:])
            Szero = cp.tile([P, P], F32)
            nc.vector.tensor_scalar(out=Szero[:], in0=ident[:], scalar1=0.0,
                                    scalar2=None, op0=AL.mult)

            def dense(xsrc_ap_of_blk, Wl, Wr, blr, dst_dram, cout, transpose_in):
                for j in range(NBLKC):
                    if transpose_in:
                        ht = dp.tile([P, P], F32, tag="ht")
                        nc.sync.dma_start(out=ht[:], in_=xsrc_ap_of_blk(j))
                        pT = ppt.tile([P, P], F32, space="PSUM")
                        nc.tensor.transpose(out=pT[:], in_=ht[:],
                                            identity=ident[:])
                        xt = dp.tile([P, P], F32, tag="xt")
                        nc.scalar.copy(xt[:], pT[:])
                    else:
                        xt = dp.tile([P, P], F32, tag="xt")
                        nc.sync.dma_start(out=xt[:], in_=xsrc_ap_of_blk(j))
                    ps = ppd.tile([P, 2 * cout], F32, space="PSUM")
                    nc.tensor.matmul(ps[:, 0:cout], lhsT=xt[:], rhs=Wl[:],
                                     start=True, stop=True)
                    nc.tensor.matmul(ps[:, cout:2 * cout], lhsT=xt[:], rhs=Wr[:],
                                     start=True, stop=True)
                    xlr = dp.tile([P, 2 * cout], F32, tag="xlr")
                    nc.vector.tensor_tensor(out=xlr[:], in0=ps[:], in1=blr[:],
                                            op=AL.add)
                    oap = dst_dram[j * 2 * P:(j + 1) * 2 * P, :].rearrange(
                        "(p two) c -> p (two c)", two=2)
                    nc.sync.dma_start(out=oap, in_=xlr[:])

            def edge_phase(table, cout, H, vV, attV, biasV, out_dram, relu,
                           out_dt=F32):
                Cc = cout // H
                # block bookkeeping per global column
                blk_of, start_c, stop_c = [], [], []
                for s in range(NBLKC):
                    for t in range(int(T_slot[s])):
                        blk_of.append(s)
                        start_c.append(t == 0)
                        stop_c.append(t == int(T_slot[s]) - 1)
                psO = None
                for g in range(Tpad // CH):
                    idxc = sp.tile([P, CH], I32, tag="idxc")
                    nc.sync.dma_start(out=idxc[:], in_=ap2d("idx")[:, g * CH:(g + 1) * CH].bitcast(I32))
                    dstc = sp.tile([P, CH], I32, tag="dstc")
                    nc.sync.dma_start(out=dstc[:], in_=ap2d("dstg")[:, g * CH:(g + 1) * CH].bitcast(I32))
                    drc = sp.tile([P, CH], F32, tag="drc")
                    nc.sync.dma_start(out=drc[:], in_=ap2d("dr")[:, g * CH:(g + 1) * CH])
                    eac = sp.tile([P, CH], F32, tag="eac")
                    nc.sync.dma_start(out=eac[:], in_=ap2d("ea")[:, g * CH:(g + 1) * CH])

                    W = cout + H          # rhs row: [scaled msg | ex]
                    # allocate at layer-1 sizes so L2 reuses the same slots;
                    # only the first CH*cout (resp. CH*W) columns are used.
                    msgA_t = chp.tile([P, CH * CO1], F32, tag="msgA")
                    m_t = chp.tile([P, CH * CO1], F32, tag="m")
                    wk_t = chp.tile([P, CH * CO1], F32, tag="wk")
                    rhs_t = chp.tile([P, CH * (CO1 + H1)], F32, tag="rhs")
                    tabs_t = chp.tile([P, CH * CO1], F32, tag="tabs")
                    msgA = msgA_t[:, 0:CH * cout]
                    m = m_t[:, 0:CH * cout]
                    wk = wk_t[:, 0:CH * cout]
                    rhs = rhs_t[:, 0:CH * W]
                    tabs = tabs_t[:, 0:CH * cout]
                    for t in range(CH):
                        nc.gpsimd.indirect_dma_start(
                            out=msgA[:, t * cout:(t + 1) * cout], out_offset=None,
                            in_=table[:, :],
                            in_offset=bass.IndirectOffsetOnAxis(ap=idxc[:, t:t + 1], axis=0))
                        nc.gpsimd.indirect_dma_start(
                            out=m[:, t * cout:(t + 1) * cout], out_offset=None,
                            in_=table[:, :],
                            in_offset=bass.IndirectOffsetOnAxis(ap=dstc[:, t:t + 1], axis=0))
                    # m = msgA + xr[dst] ; m += ea * vV (broadcast)
                    nc.vector.tensor_tensor(out=m[:], in0=m[:], in1=msgA[:], op=AL.add)
                    eb = eac[:].rearrange("p (t o) -> p t o", o=1)
                    vb = vV[:].rearrange("p (o c) -> p o c", o=1)
                    ebb, vbb = bass.broadcast_tensor_aps(eb, vb)
                    mv = m[:].rearrange("p (t c) -> p t c", t=CH)
                    wkv = wk[:].rearrange("p (t c) -> p t c", t=CH)
                    nc.vector.tensor_tensor(out=wkv, in0=ebb, in1=vbb, op=AL.mult)
                    nc.vector.tensor_tensor(out=m[:], in0=m[:], in1=wk[:], op=AL.add)
                    # tabs = |m| ; q = m*att ; lin = reduce ; u = |m|*att ; ur
                    nc.scalar.activation(tabs[:], m[:], AF.Abs)
                    av = attV[:].rearrange("p (o c) -> p o c", o=1)
                    _, avb = bass.broadcast_tensor_aps(mv, av)
                    nc.vector.tensor_tensor(out=wkv, in0=mv, in1=avb, op=AL.mult)
                    lin = sp.tile([P, CH * H], F32, tag="lin")
                    nc.vector.tensor_reduce(out=lin[:],
                                            in_=wk[:].rearrange("p (th c) -> p th c", c=Cc),
                                            axis=mybir.AxisListType.X, op=AL.add)
                    tv = tabs[:].rearrange("p (t c) -> p t c", t=CH)
                    nc.vector.tensor_tensor(out=wkv, in0=tv, in1=avb, op=AL.mult)
                    ur = sp.tile([P, CH * H], F32, tag="ur")
                    nc.vector.tensor_reduce(out=ur[:],
                                            in_=wk[:].rearrange("p (th c) -> p th c", c=Cc),
                                            axis=mybir.AxisListType.X, op=AL.add)
                    logit = sp.tile([P, CH * H], F32, tag="logit")
                    nc.vector.tensor_scalar(out=logit[:], in0=lin[:], scalar1=0.6,
                                            scalar2=None, op0=AL.mult)
                    nc.vector.scalar_tensor_tensor(out=logit[:], in0=ur[:], scalar=0.4,
                                                   in1=logit[:], op0=AL.mult, op1=AL.add)
                    ex = sp.tile([P, CH * H], F32, tag="ex")
                    nc.scalar.activation(ex[:], logit[:], AF.Exp)
                    # copy ex into the tail H columns of each tile's rhs slot
                    exdst = rhs[:].rearrange("p (t w) -> p t w", w=W)[:, :, cout:cout + H]
                    nc.scalar.copy(exdst, ex[:].rearrange("p (t h) -> p t h", h=H))

                    for t in range(CH):
                        c = g * CH + t
                        s = blk_of[c]
                        if start_c[c]:
                            psO = ppo.tile([P, W], F32, space="PSUM")
                            # the first start=True accumulation is dropped by
                            # HW; absorb it with a zero matmul per block.
                            nc.tensor.matmul(psO[:], lhsT=Szero[:],
                                             rhs=C["blr1"][:, 0:W],
                                             start=True, stop=False)
                        S01 = tp.tile([P, P], F32, tag="S01")
                        nc.vector.tensor_scalar(out=S01[:], in0=C["iotaV"][:],
                                                scalar1=drc[:, t:t + 1], scalar2=None,
                                                op0=AL.is_equal)
                        # scaled = msgA_tile * ex (per-head broadcast) -> rhs slot
                        sc = rhs[:, t * W:t * W + cout].rearrange(
                            "p (h c) -> p h c", h=H)
                        mg = msgA[:, t * cout:(t + 1) * cout].rearrange(
                            "p (h c) -> p h c", h=H)
                        eview = ex[:, t * H:(t + 1) * H].rearrange("p (h o) -> p h o", o=1)
                        _, evb = bass.broadcast_tensor_aps(mg, eview)
                        nc.vector.tensor_tensor(out=sc, in0=mg, in1=evb, op=AL.mult)
                        nc.tensor.matmul(psO[:], lhsT=S01[:],
                                         rhs=rhs[:, t * W:(t + 1) * W],
                                         start=False, stop=bool(stop_c[c]))
                        if stop_c[c]:
                            den = epp.tile([P, H], F32, tag="den")
                            nc.vector.tensor_scalar_max(den[:], psO[:, cout:cout + H], 1e-30)
                            dinv = epp.tile([P, H], F32, tag="dinv")
                            nc.vector.reciprocal(dinv[:], den[:])
                            hsb = epp.tile([P, cout], F32, tag="hsb")
                            hv = hsb[:].rearrange("p (h c) -> p h c", h=H)
                            pv = psO[:, 0:cout].rearrange("p (h c) -> p h c", h=H)
                            dv = dinv[:].rearrange("p (h o) -> p h o", o=1)
                            _, dvb = bass.broadcast_tensor_aps(pv, dv)
                            nc.vector.tensor_tensor(out=hv, in0=pv, in1=dvb, op=AL.mult)
                            hfin = epp.tile([P, cout], out_dt, tag="hfin")
                            nc.vector.tensor_tensor(out=hfin[:], in0=hsb[:], in1=biasV[:],
                                                    op=AL.add)
                            if relu:
                                nc.vector.tensor_scalar_max(hfin[:], hfin[:], 0.0)
                            nc.sync.dma_start(out=out_dram[s * P:(s + 1) * P, :],
                                              in_=hfin[:])

            # ---------- layer 1 ----------
            dense(lambda j: ap2d("xT")[:, j * P:(j + 1) * P],
                  C["W1l"], C["W1r"], C["blr1"], XLR1_loc, CO1, transpose_in=False)
            nc.gpsimd.collective_compute(
                "AllGather", AL.bypass, replica_groups=[list(range(NCORES))],
                ins=[XLR1_loc[:, :]], outs=[XLR1[:, :]])
            edge_phase(XLR1, CO1, H1, C["vV1"], C["attV1"], C["b1"], HL1, relu=True)
            # ---------- layer 2 ----------
            dense(lambda j: HL1[j * P:(j + 1) * P, :],
                  C["W2l"], C["W2r"], C["blr2"], XLR2_loc, CO2, transpose_in=True)
            nc.gpsimd.collective_compute(
                "AllGather", AL.bypass, replica_groups=[list(range(NCORES))],
                ins=[XLR2_loc[:, :]], outs=[XLR2[:, :]])
            edge_phase(XLR2, CO2, H2, C["vV2"], C["attV2"], C["b2"], out_d,
                       relu=False, out_dt=BF16)
    return nc, offs, NW


def _make_consts(W1_l, b1_l, W1_r, b1_r, W1_e, att1, bias1,
                 W2_l, b2_l, W2_r, b2_r, W2_e, att2, bias2):
    iota = np.repeat(np.arange(P, dtype=np.float32)[None, :], P, axis=0)
    return [
        iota,
        np.asarray(W1_l, np.float32), np.asarray(W1_r, np.float32),
        _rep(np.concatenate([np.asarray(b1_l).ravel(), np.asarray(b1_r).ravel()])),
        _rep(np.asarray(W1_e).ravel()), _rep(np.asarray(att1).ravel()),
        _rep(np.asarray(bias1).ravel()),
        np.asarray(W2_l, np.float32), np.asarray(W2_r, np.float32),
        _rep(np.concatenate([np.asarray(b2_l).ravel(), np.asarray(b2_r).ravel()])),
        _rep(np.asarray(W2_e).ravel()), _rep(np.asarray(att2).ravel()),
        _rep(np.asarray(bias2).ravel()),
    ]


def kernel(x, edge_index, edge_attr,
           W1_l, b1_l, W1_r, b1_r, W1_e, att1, bias1,
           W2_l, b2_l, W2_r, b2_r, W2_e, att2, bias2):
    _mark("kernel start")
    x = np.asarray(x, np.float32)
    prep = _prep_edges(edge_index, edge_attr)
    Tpad = prep["Tpad"]
    _mark("prep done")

    nc, offs, NW = _build_kernel(Tpad, prep["T_slot"])
    _mark("build done")

    consts = _make_consts(W1_l, b1_l, W1_r, b1_r, W1_e, att1, bias1,
                          W2_l, b2_l, W2_r, b2_r, W2_e, att2, bias2)
    cvec = np.concatenate([c.ravel() for c in consts]).astype(np.float32)

    xpad = np.zeros((NTOT, F_IN), np.float32)
    xpad[:N_NODES] = x
    xT = np.ascontiguousarray(xpad.T)          # [128, NTOT]

    in_maps = []
    for k in range(NCORES):
        blobv = np.empty((1, NW), np.float32)
        o = 0
        blobv[0, o:o + cvec.size] = cvec
        o += cvec.size
        blobv[0, o:o + P * NLOC] = xT[:, k * NLOC:(k + 1) * NLOC].ravel()
        o += P * NLOC
        for arr in (prep["idx_st"][k].view(np.float32),
                    prep["dst_st"][k].view(np.float32),
                    prep["dr_st"][k], prep["ea_st"][k]):
            blobv[0, o:o + P * Tpad] = arr.ravel()
            o += P * Tpad
        assert o == NW
        in_maps.append({"blob": blobv})
    _mark("blobs packed")

    res = run_bass_kernel_spmd(nc, in_maps, core_ids=list(range(NCORES)))
    _mark("spmd run done")
    out = np.concatenate([res.results[k]["out"] for k in range(NCORES)], axis=0)
    _mark("fetch done")
    return out[:N_NODES].astype(np.float32)


# revision 42
# speedup vs baseline: 22.5218x; 1.9240x over previous
"""GATv2 (2-layer) Trainium2 Bass kernel, 8-core SPMD, single fused NEFF.

Wall-clock-oriented design (device exec is ~0.1s; build/compile/transfer
dominate):
- ONE kernel for both layers; h is exchanged on-device with an AllGather
  collective (no inter-layer host round trip, one compile, one launch).
- Uniform node sharding: 784 blocks of 128 nodes, 98 blocks per core, so
  AllGather slices concatenate into global node order and one edge-index
  stream serves both layers.
- xl/xr tables are row-interleaved ([2N, C]: row 2n = xl_n, 2n+1 = xr_n),
  so gathers use indices 2*src and 2*dst+1 into the same table.
- All per-core inputs are packed into a single f32 blob (one sharded
  transfer); int32 stream regions are viewed via AP.bitcast.
- Edge phase: per 128-edge tile only 5 instructions (2 gathers, one-hot
  build, exp-prescale into an rhs buffer that also carries the exp column,
  and ONE aggregation matmul over [cout+H] columns); the logit pipeline is
  batched over CH=32 tiles with broadcast APs.
- Segment softmax without max subtraction (logits are O(1); exact enough),
  denominator applied after aggregation. leaky_relu via 0.6x + 0.4|x|.
- Final output in bf16 (value-proportional rounding keeps relative error
  safe); inputs/tables stay f32 (bf16 there creates absolute-scale errors
  that blow up the relative-error metric at near-zero outputs).
"""

import json
import os
import sys
import threading
import time as _time
import numpy as np

# Smaller/faster NEFF packaging (no debug info); read by walrus arg builder.
os.environ.setdefault("CONCOURSE_SCRUB_NEFF_DEBUG_INFO", "1")

_T0 = _time.time()


def _mark(msg):
    print(f"[kernel +{_time.time() - _T0:6.2f}s] {msg}", file=sys.stderr, flush=True)

import concourse.bass as bass
import concourse.mybir as mybir
from concourse.tile import TileContext, ScopedClock
from concourse.bass_utils import run_bass_kernel_spmd
from concourse.masks import make_identity

# ----------------------------------------------------------------------------
# Workarounds for the walrus build in this container: at most ONE sync-wait
# per instruction. Extra waits are peeled onto NoOps inserted just before.
# ----------------------------------------------------------------------------
_MAXW = 1
_split_counter = [0]


def _patched_drain_and_barrier(self, tick_clock, wait_clock):
    d0 = self.nc.sync.drain()
    wait_clock.add_sem_waits(d0.ins, ScopedClock({None: tick_clock.global_clock}))
    waits = list(d0.ins.sync_info.on_wait)
    if len(waits) > _MAXW:
        del d0.ins.sync_info.on_wait[_MAXW:]
        rest = waits[_MAXW:]
        for i in range(0, len(rest), _MAXW):
            d = self.nc.sync.drain()
            if d.ins.sync_info is None:
                d.ins.sync_info = mybir.SyncInfo(on_update=[], on_wait=[])
            d.ins.sync_info.on_wait.extend(rest[i:i + _MAXW])
    self.nc.all_engine_barrier()
    popped = self.nc._tile_sem_poison_stack.pop()
    assert popped is self._sem_poison
    self.nc.clear_and_free_semaphores(list(self.sems.allocated().values()))
    self.nc.all_engine_barrier()


def _fix_bir_json(data: bytes) -> bytes:
    try:
        import orjson
        _loads, _dumps = orjson.loads, lambda m: orjson.dumps(m)
    except ImportError:
        _loads, _dumps = json.loads, lambda m: json.dumps(m).encode()
    m = _loads(data)
    changed = False
    for f in m.get("functions", []):
        for b in f.get("blocks", []):
            insts = b.get("instructions")
            if not insts:
                continue
            out = []
            for inst in insts:
                si = inst.get("sync_info") or {}
                waits = si.get("on_wait") or []
                if len(waits) > 1:
                    for w in waits[:-1]:
                        _split_counter[0] += 1
                        out.append({
                            "name": f"I-sw{_split_counter[0]}",
                            "opcode": "NoOp",
                            "engine": inst.get("engine"),
                            "ins": [], "outs": [],
                            "sync_info": {"on_update": [], "on_wait": [w]},
                        })
                    si["on_wait"] = [waits[-1]]
                    changed = True
                out.append(inst)
            b["instructions"] = out
    if not changed:
        return data
    return _dumps(m)


def _install_fixes():
    TileContext._drain_and_barrier = _patched_drain_and_barrier
    if not getattr(bass.Bass, "_tilefix_json", False):
        orig = bass.Bass.to_json_bytes

        def to_json_bytes(self, *a, **k):
            return _fix_bir_json(orig(self, *a, **k))

        bass.Bass.to_json_bytes = to_json_bytes
        bass.Bass._tilefix_json = True


_install_fixes()

# ----------------------------------------------------------------------------
N_NODES = 100_000
N_EDGES = 1_600_000
F_IN = 128
H1, C1 = 2, 64
H2, C2 = 1, 64
CO1, CO2 = H1 * C1, H2 * C2            # 128, 64
NCORES = 8
P = 128
NBLKC = 98                              # blocks per core
NLOC = NBLKC * P                        # 12544 nodes per core
NTOT = NCORES * NLOC                    # 100352 padded nodes
CH = 32                                 # tiles per merged logit chunk
F32 = mybir.dt.float32
BF16 = mybir.dt.bfloat16
I32 = mybir.dt.int32
AL = mybir.AluOpType
AF = mybir.ActivationFunctionType


def _rep(v):
    v = np.asarray(v, np.float32).reshape(1, -1)
    return np.ascontiguousarray(np.repeat(v, P, axis=0))


def _prep_edges(edge_index, edge_attr):
    """Sort edges by dst; build per-core [128, Tpad] streams (vectorized)."""
    src = np.asarray(edge_index[0], np.int64)
    dst = np.asarray(edge_index[1], np.int64)
    E = src.shape[0]
    order = np.argsort(dst, kind="stable")
    src_s = src[order].astype(np.int64)
    dst_s = dst[order].astype(np.int64)
    ea_s = np.asarray(edge_attr, np.float32).reshape(-1)[order]
    blk = (dst_s >> 7).astype(np.int64)            # global block 0..781
    cnt = np.bincount(blk, minlength=NCORES * NBLKC)
    T_slot = np.maximum((cnt.reshape(NCORES, NBLKC) + P - 1) // P, 1).max(axis=0)
    col0 = np.zeros(NBLKC + 1, np.int64)
    col0[1:] = np.cumsum(T_slot)
    sumT = int(col0[-1])
    Tpad = ((sumT + CH - 1) // CH) * CH
    T_slot = T_slot.copy()
    T_slot[-1] += Tpad - sumT                      # tail pad columns absorb
    runstart = np.zeros(NCORES * NBLKC + 1, np.int64)
    runstart[1:] = np.cumsum(cnt)
    rank = np.arange(E, dtype=np.int64) - runstart[blk]
    core = blk // NBLKC
    slot = blk - core * NBLKC
    col = col0[slot] + (rank >> 7)
    row = rank & 127

    idx_st = np.zeros((NCORES, P, Tpad), np.int32)
    dst_st = np.ones((NCORES, P, Tpad), np.int32)
    dr_st = np.full((NCORES, P, Tpad), -1.0, np.float32)
    ea_st = np.zeros((NCORES, P, Tpad), np.float32)
    idx_st[core, row, col] = (2 * src_s).astype(np.int32)
    dst_st[core, row, col] = (2 * dst_s + 1).astype(np.int32)
    dr_st[core, row, col] = (dst_s & 127).astype(np.float32)
    ea_st[core, row, col] = ea_s
    return dict(Tpad=Tpad, T_slot=T_slot.astype(np.int64), idx_st=idx_st,
                dst_st=dst_st, dr_st=dr_st, ea_st=ea_st)


def _build_kernel(Tpad, T_slot):
    nc = bass.Bass()

    # ---- blob layout (element offsets into the per-core [1, NW] f32 blob)
    widths = dict(iotaV=P, W1l=CO1, W1r=CO1, blr1=2 * CO1, vV1=CO1,
                  attV1=CO1, b1=CO1, W2l=CO2, W2r=CO2, blr2=2 * CO2,
                  vV2=CO2, attV2=CO2, b2=CO2)
    offs = {}
    off = 0
    for k, w in widths.items():
        offs[k] = off
        off += P * w
    offs["xT"] = off
    off += P * NLOC
    for s in ("idx", "dstg", "dr", "ea"):
        offs[s] = off
        off += P * Tpad
    NW = off

    blob = nc.dram_tensor("blob", [1, NW], F32, kind="ExternalInput")
    out_d = nc.dram_tensor("out", [NLOC, CO2], BF16, kind="ExternalOutput")
    XLR1_loc = nc.dram_tensor("XLR1_loc", [2 * NLOC, CO1], F32)
    XLR1 = nc.dram_tensor("XLR1", [2 * NTOT, CO1], F32)
    HL1 = nc.dram_tensor("HL1", [NLOC, CO1], F32)
    XLR2_loc = nc.dram_tensor("XLR2_loc", [2 * NLOC, CO2], F32)
    XLR2 = nc.dram_tensor("XLR2", [2 * NTOT, CO2], F32)

    def ap2d(name, w=None):
        o, tw = offs[name], widths.get(name, Tpad if name in
                                       ("idx", "dstg", "dr", "ea") else None)
        if name == "xT":
            tw = NLOC
        if w is None:
            w = tw
        return blob[0:1, o:o + P * tw].rearrange("o (p w) -> (o p) w", p=P)

    with TileContext(nc) as tc:
        with (
            tc.tile_pool(name="const", bufs=1) as cp,
            tc.tile_pool(name="dense", bufs=3) as dp,
            tc.tile_pool(name="st", bufs=3) as sp,
            tc.tile_pool(name="chunk", bufs=2) as chp,
            tc.tile_pool(name="tile", bufs=6) as tp,
            tc.tile_pool(name="ep", bufs=2) as epp,
            tc.tile_pool(name="pd", bufs=2, space="PSUM") as ppd,
            tc.tile_pool(name="po", bufs=2, space="PSUM") as ppo,
            tc.tile_pool(name="pt", bufs=2, space="PSUM") as ppt,
        ):
            C = {}
            for k, w in widths.items():
                t = cp.tile([P, w], F32, tag=f"c_{k}")
                nc.sync.dma_start(out=t[:], in_=ap2d(k))
                C[k] = t
            ident = cp.tile([P, P], F32)
            make_identity(nc, ident[:])
            Szero = cp.tile([P, P], F32)
            nc.vector.tensor_scalar(out=Szero[:], in0=ident[:], scalar1=0.0,
                                    scalar2=None, op0=AL.mult)

            def dense(xsrc_ap_of_blk, Wl, Wr, blr, dst_dram, cout, transpose_in):
                for j in range(NBLKC):
                    if transpose_in:
                        ht = dp.tile([P, P], F32, tag="ht")
                        nc.sync.dma_start(out=ht[:], in_=xsrc_ap_of_blk(j))
                        pT = ppt.tile([P, P], F32, space="PSUM")
                        nc.tensor.transpose(out=pT[:], in_=ht[:],
                                            identity=ident[:])
                        xt = dp.tile([P, P], F32, tag="xt")
                        nc.scalar.copy(xt[:], pT[:])
                    else:
                        xt = dp.tile([P, P], F32, tag="xt")
                        nc.sync.dma_start(out=xt[:], in_=xsrc_ap_of_blk(j))
                    ps = ppd.tile([P, 2 * cout], F32, space="PSUM")
                    nc.tensor.matmul(ps[:, 0:cout], lhsT=xt[:], rhs=Wl[:],
                                     start=True, stop=True)
                    nc.tensor.matmul(ps[:, cout:2 * cout], lhsT=xt[:], rhs=Wr[:],
                                     start=True, stop=True)
                    xlr = dp.tile([P, 2 * cout], F32, tag="xlr")
                    nc.vector.tensor_tensor(out=xlr[:], in0=ps[:], in1=blr[:],
                                            op=AL.add)
                    oap = dst_dram[j * 2 * P:(j + 1) * 2 * P, :].rearrange(
                        "(p two) c -> p (two c)", two=2)
                    nc.sync.dma_start(out=oap, in_=xlr[:])

            def edge_phase(table, cout, H, vV, attV, biasV, out_dram, relu,
                           out_dt=F32):
                Cc = cout // H
                # block bookkeeping per global column
                blk_of, start_c, stop_c = [], [], []
                for s in range(NBLKC):
                    for t in range(int(T_slot[s])):
                        blk_of.append(s)
                        start_c.append(t == 0)
                        stop_c.append(t == int(T_slot[s]) - 1)
                psO = None
                for g in range(Tpad // CH):
                    idxc = sp.tile([P, CH], I32, tag="idxc")
                    nc.sync.dma_start(out=idxc[:], in_=ap2d("idx")[:, g * CH:(g + 1) * CH].bitcast(I32))
                    dstc = sp.tile([P, CH], I32, tag="dstc")
                    nc.sync.dma_start(out=dstc[:], in_=ap2d("dstg")[:, g * CH:(g + 1) * CH].bitcast(I32))
                    drc = sp.tile([P, CH], F32, tag="drc")
                    nc.sync.dma_start(out=drc[:], in_=ap2d("dr")[:, g * CH:(g + 1) * CH])
                    eac = sp.tile([P, CH], F32, tag="eac")
                    nc.sync.dma_start(out=eac[:], in_=ap2d("ea")[:, g * CH:(g + 1) * CH])

                    W = cout + H          # rhs row: [scaled msg | ex]
                    # allocate at layer-1 sizes so L2 reuses the same slots;
                    # only the first CH*cout (resp. CH*W) columns are used.
                    msgA_t = chp.tile([P, CH * CO1], F32, tag="msgA")
                    m_t = chp.tile([P, CH * CO1], F32, tag="m")
                    wk_t = chp.tile([P, CH * CO1], F32, tag="wk")
                    rhs_t = chp.tile([P, CH * (CO1 + H1)], F32, tag="rhs")
                    tabs_t = chp.tile([P, CH * CO1], F32, tag="tabs")
                    msgA = msgA_t[:, 0:CH * cout]
                    m = m_t[:, 0:CH * cout]
                    wk = wk_t[:, 0:CH * cout]
                    rhs = rhs_t[:, 0:CH * W]
                    tabs = tabs_t[:, 0:CH * cout]
                    for t in range(CH):
                        nc.gpsimd.indirect_dma_start(
                            out=msgA[:, t * cout:(t + 1) * cout], out_offset=None,
                            in_=table[:, :],
                            in_offset=bass.IndirectOffsetOnAxis(ap=idxc[:, t:t + 1], axis=0))
                        nc.gpsimd.indirect_dma_start(
                            out=m[:, t * cout:(t + 1) * cout], out_offset=None,
                            in_=table[:, :],
                            in_offset=bass.IndirectOffsetOnAxis(ap=dstc[:, t:t + 1], axis=0))
                    # m = msgA + xr[dst] ; m += ea * vV (broadcast)
                    nc.vector.tensor_tensor(out=m[:], in0=m[:], in1=msgA[:], op=AL.add)
                    eb = eac[:].rearrange("p (t o) -> p t o", o=1)
                    vb = vV[:].rearrange("p (o c) -> p o c", o=1)
                    ebb, vbb = bass.broadcast_tensor_aps(eb, vb)
                    mv = m[:].rearrange("p (t c) -> p t c", t=CH)
                    wkv = wk[:].rearrange("p (t c) -> p t c", t=CH)
                    nc.vector.tensor_tensor(out=wkv, in0=ebb, in1=vbb, op=AL.mult)
                    nc.vector.tensor_tensor(out=m[:], in0=m[:], in1=wk[:], op=AL.add)
                    # tabs = |m| ; q = m*att ; lin = reduce ; u = |m|*att ; ur
                    nc.scalar.activation(tabs[:], m[:], AF.Abs)
                    av = attV[:].rearrange("p (o c) -> p o c", o=1)
                    _, avb = bass.broadcast_tensor_aps(mv, av)
                    nc.vector.tensor_tensor(out=wkv, in0=mv, in1=avb, op=AL.mult)
                    lin = sp.tile([P, CH * H], F32, tag="lin")
                    nc.vector.tensor_reduce(out=lin[:],
                                            in_=wk[:].rearrange("p (th c) -> p th c", c=Cc),
                                            axis=mybir.AxisListType.X, op=AL.add)
                    tv = tabs[:].rearrange("p (t c) -> p t c", t=CH)
                    nc.vector.tensor_tensor(out=wkv, in0=tv, in1=avb, op=AL.mult)
                    ur = sp.tile([P, CH * H], F32, tag="ur")
                    nc.vector.tensor_reduce(out=ur[:],
                                            in_=wk[:].rearrange("p (th c) -> p th c", c=Cc),
                                            axis=mybir.AxisListType.X, op=AL.add)
                    logit = sp.tile([P, CH * H], F32, tag="logit")
                    nc.vector.tensor_scalar(out=logit[:], in0=lin[:], scalar1=0.6,
                                            scalar2=None, op0=AL.mult)
                    nc.vector.scalar_tensor_tensor(out=logit[:], in0=ur[:], scalar=0.4,
                                                   in1=logit[:], op0=AL.mult, op1=AL.add)
                    ex = sp.tile([P, CH * H], F32, tag="ex")
                    nc.scalar.activation(ex[:], logit[:], AF.Exp)
                    # copy ex into the tail H columns of each tile's rhs slot
                    exdst = rhs[:].rearrange("p (t w) -> p t w", w=W)[:, :, cout:cout + H]
                    nc.scalar.copy(exdst, ex[:].rearrange("p (t h) -> p t h", h=H))

                    for t in range(CH):
                        c = g * CH + t
                        s = blk_of[c]
                        if start_c[c]:
                            psO = ppo.tile([P, W], F32, space="PSUM")
                            # the first start=True accumulation is dropped by
                            # HW; absorb it with a zero matmul per block.
                            nc.tensor.matmul(psO[:], lhsT=Szero[:],
                                             rhs=C["blr1"][:, 0:W],
                                             start=True, stop=False)
                        S01 = tp.tile([P, P], F32, tag="S01")
                        nc.vector.tensor_scalar(out=S01[:], in0=C["iotaV"][:],
                                                scalar1=drc[:, t:t + 1], scalar2=None,
                                                op0=AL.is_equal)
                        # scaled = msgA_tile * ex (per-head broadcast) -> rhs slot
                        sc = rhs[:, t * W:t * W + cout].rearrange(
                            "p (h c) -> p h c", h=H)
                        mg = msgA[:, t * cout:(t + 1) * cout].rearrange(
                            "p (h c) -> p h c", h=H)
                        eview = ex[:, t * H:(t + 1) * H].rearrange("p (h o) -> p h o", o=1)
                        _, evb = bass.broadcast_tensor_aps(mg, eview)
                        nc.vector.tensor_tensor(out=sc, in0=mg, in1=evb, op=AL.mult)
                        nc.tensor.matmul(psO[:], lhsT=S01[:],
                                         rhs=rhs[:, t * W:(t + 1) * W],
                                         start=False, stop=bool(stop_c[c]))
                        if stop_c[c]:
                            den = epp.tile([P, H], F32, tag="den")
                            nc.vector.tensor_scalar_max(den[:], psO[:, cout:cout + H], 1e-30)
                            dinv = epp.tile([P, H], F32, tag="dinv")
                            nc.vector.reciprocal(dinv[:], den[:])
                            hsb = epp.tile([P, cout], F32, tag="hsb")
                            hv = hsb[:].rearrange("p (h c) -> p h c", h=H)
                            pv = psO[:, 0:cout].rearrange("p (h c) -> p h c", h=H)
                            dv = dinv[:].rearrange("p (h o) -> p h o", o=1)
                            _, dvb = bass.broadcast_tensor_aps(pv, dv)
                            nc.vector.tensor_tensor(out=hv, in0=pv, in1=dvb, op=AL.mult)
                            hfin = epp.tile([P, cout], out_dt, tag="hfin")
                            nc.vector.tensor_tensor(out=hfin[:], in0=hsb[:], in1=biasV[:],
                                                    op=AL.add)
                            if relu:
                                nc.vector.tensor_scalar_max(hfin[:], hfin[:], 0.0)
                            nc.sync.dma_start(out=out_dram[s * P:(s + 1) * P, :],
                                              in_=hfin[:])

            # ---------- layer 1 ----------
            dense(lambda j: ap2d("xT")[:, j * P:(j + 1) * P],
                  C["W1l"], C["W1r"], C["blr1"], XLR1_loc, CO1, transpose_in=False)
            nc.gpsimd.collective_compute(
                "AllGather", AL.bypass, replica_groups=[list(range(NCORES))],
                ins=[XLR1_loc[:, :]], outs=[XLR1[:, :]])
            edge_phase(XLR1, CO1, H1, C["vV1"], C["attV1"], C["b1"], HL1, relu=True)
            # ---------- layer 2 ----------
            dense(lambda j: HL1[j * P:(j + 1) * P, :],
                  C["W2l"], C["W2r"], C["blr2"], XLR2_loc, CO2, transpose_in=True)
            nc.gpsimd.collective_compute(
                "AllGather", AL.bypass, replica_groups=[list(range(NCORES))],
                ins=[XLR2_loc[:, :]], outs=[XLR2[:, :]])
            edge_phase(XLR2, CO2, H2, C["vV2"], C["attV2"], C["b2"], out_d,
                       relu=False, out_dt=BF16)
    return nc, offs, NW


def _make_consts(W1_l, b1_l, W1_r, b1_r, W1_e, att1, bias1,
                 W2_l, b2_l, W2_r, b2_r, W2_e, att2, bias2):
    iota = np.repeat(np.arange(P, dtype=np.float32)[None, :], P, axis=0)
    return [
        iota,
        np.asarray(W1_l, np.float32), np.asarray(W1_r, np.float32),
        _rep(np.concatenate([np.asarray(b1_l).ravel(), np.asarray(b1_r).ravel()])),
        _rep(np.asarray(W1_e).ravel()), _rep(np.asarray(att1).ravel()),
        _rep(np.asarray(bias1).ravel()),
        np.asarray(W2_l, np.float32), np.asarray(W2_r, np.float32),
        _rep(np.concatenate([np.asarray(b2_l).ravel(), np.asarray(b2_r).ravel()])),
        _rep(np.asarray(W2_e).ravel()), _rep(np.asarray(att2).ravel()),
        _rep(np.asarray(bias2).ravel()),
    ]


def _warm_devices():
    """Establish the axon/PJRT session (network handshakes) while the main
    thread does CPU-bound prep/build; device_put releases the GIL."""
    try:
        import jax
        d = jax.devices()
        jax.device_put(np.zeros((8, 8), np.float32), d[0]).block_until_ready()
    except Exception:
        pass


def kernel(x, edge_index, edge_attr,
           W1_l, b1_l, W1_r, b1_r, W1_e, att1, bias1,
           W2_l, b2_l, W2_r, b2_r, W2_e, att2, bias2):
    _mark("kernel start")
    warm = threading.Thread(target=_warm_devices, daemon=True)
    warm.start()
    x = np.asarray(x, np.float32)
    prep = _prep_edges(edge_index, edge_attr)
    Tpad = prep["Tpad"]
    _mark("prep done")

    nc, offs, NW = _build_kernel(Tpad, prep["T_slot"])
    _mark("build done")

    consts = _make_consts(W1_l, b1_l, W1_r, b1_r, W1_e, att1, bias1,
                          W2_l, b2_l, W2_r, b2_r, W2_e, att2, bias2)
    cvec = np.concatenate([c.ravel() for c in consts]).astype(np.float32)

    xpad = np.zeros((NTOT, F_IN), np.float32)
    xpad[:N_NODES] = x
    xT = np.ascontiguousarray(xpad.T)          # [128, NTOT]

    in_maps = []
    for k in range(NCORES):
        blobv = np.empty((1, NW), np.float32)
        o = 0
        blobv[0, o:o + cvec.size] = cvec
        o += cvec.size
        blobv[0, o:o + P * NLOC] = xT[:, k * NLOC:(k + 1) * NLOC].ravel()
        o += P * NLOC
        for arr in (prep["idx_st"][k].view(np.float32),
                    prep["dst_st"][k].view(np.float32),
                    prep["dr_st"][k], prep["ea_st"][k]):
            blobv[0, o:o + P * Tpad] = arr.ravel()
            o += P * Tpad
        assert o == NW
        in_maps.append({"blob": blobv})
    _mark("blobs packed")
    warm.join(timeout=120)

    res = run_bass_kernel_spmd(nc, in_maps, core_ids=list(range(NCORES)))
    _mark("spmd run done")
    out = np.concatenate([res.results[k]["out"] for k in range(NCORES)], axis=0)
    _mark("fetch done")
    return out[:N_NODES].astype(np.float32)
